# revision 16
# baseline (speedup 1.0000x reference)
"""CrystalLinear TRN2 kernel, v4: bitcast ternary decode + 2x4 sharding.

Grid: M split 2 x N split 4 (core c = mh*4 + nq). Per core mq=512,
nloc=2048, kq=8192.

Weight decode: the 2-bit fields live in int16 halves of the packed
words. For shift s in 0..3, (word >> 2s) & 0x0303 isolates fields s and
s+4; adding 0x5050 turns each byte into the fp8e4m3 bit pattern of
(8 + w) (values {8,9,11} share one exponent block, so the fp8 value is
linear in the low bit field). A bitcast to fp8 yields matmul-ready
DoubleRowSwInterleave stationary pairs - the whole decode is two
4x-mode DVE int16 ops per field pair, no ACT converts.

The matmul computes sum_k (8 + w_k) x8_k. The 8*sum(x8) offset is
removed exactly with an all-ones-stationary DoubleRow matmul (S8), and
the fp8-quantization correction uses the fp16-precision column sum S16
(Pool pair adds + DVE pair tree):
    y = psum + bias + (4/3)*S16 - (28/3)*S8
      = sum w x8 + (4/3)(S16 - S8) + bias      (same numerics as v3).

PSUM: 8 banks of [128,512]f32. Units = 16 output row-blocks + S8, run
in waves (7+S8 / 8 / 1) so <= 8 banks are live; each unit is
partial-evacuated by ACT (Identity: psum + per-partition bias -> SBUF
f32) so banks free without waiting for the correction; DVE/Pool then
add rs and round to fp16.
"""

import sys

sys.path.insert(0, "/opt/trn_rl_repo")

import numpy as np

M_FULL, K_FULL, N_FULL = 1024, 8192, 8192
NCORES = 8
MSPLIT, NSPLIT = 2, 4
MQ = M_FULL // MSPLIT          # 512
NLOC = N_FULL // NSPLIT        # 2048
KQ = K_FULL                    # 8192
NW = KQ // 16                  # 512 packed words per output row
NMT = NW // 128                # 4 word-tiles
NJT = NLOC // 128              # 16 output row-blocks
NKP = KQ // 256                # 32 k-pair steps
POOL_S_PAIRS = 16              # pair-sums chained on Pool; rest chained on DVE
DVE_X_PERIOD = 4               # every 4th pair converts fp16->fp8 on DVE

_PROGRAM_CACHE = {}

# plane order: for s in 0..3, h in 0..1 the k-pair is fields
# (8h+s, 8h+s+4); host stores planes in this (slot) order.
_FIELD_ORDER = [
    f for s in range(4) for h in range(2) for f in (8 * h + s, 8 * h + s + 4)
]


def _build_program(reps=1):
    import concourse.mybir as mybir
    from concourse import bacc, bass_isa
    from concourse.tile import TileContext

    nc = bacc.Bacc(trn_type="TRN2", enable_partition_id=False)
    d_xt = nc.dram_tensor("xt", [KQ, MQ], mybir.dt.float16, kind="ExternalInput")
    d_ht = nc.dram_tensor("ht", [NW, 2 * NLOC], mybir.dt.int16, kind="ExternalInput")
    d_bias = nc.dram_tensor("bias", [128, NJT], mybir.dt.float32, kind="ExternalInput")
    d_out = nc.dram_tensor("out", [NLOC, MQ], mybir.dt.float16, kind="ExternalOutput")

    # waves of units; S8 rides in wave 0 (7 + 1 = 8 psum banks)
    WAVES = [list(range(0, 7)) + ["S8"], list(range(7, 15)), [15]]
    WAVE_COLS = [(0, 7 * 128), (7 * 128, 15 * 128), (15 * 128, 16 * 128)]

    C43 = float(np.float32(4.0) / np.float32(3.0))
    C283 = float(np.float32(28.0) / np.float32(3.0))

    with TileContext(nc) as tc:
        with (
            tc.tile_pool(name="res", bufs=1) as res,
            tc.tile_pool(name="x8p", bufs=2) as x8p,
            tc.tile_pool(name="acc", bufs=6) as accp,
            tc.tile_pool(name="xs", bufs=7) as xsp,
            tc.tile_pool(name="tp", bufs=5) as tpp,
            tc.tile_pool(name="tw", bufs=4) as twp,
            tc.tile_pool(name="ps", bufs=8, space="PSUM") as psp,
            tc.tile_pool(name="pa", bufs=4) as pap,
            tc.tile_pool(name="ot", bufs=3) as otp,
        ):
            # one-time loads (amortized across reps, like v3)
            ht_sb = res.tile([128, NMT, 2, NLOC], mybir.dt.int16)
            for mt in range(NMT):
                for h in range(2):
                    nc.sync.dma_start(
                        ht_sb[:, mt, h, :],
                        d_ht[mt * 128 : (mt + 1) * 128, h * NLOC : (h + 1) * NLOC],
                    )
            bias_sb = res.tile([128, NJT], mybir.dt.float32)
            nc.sync.dma_start(bias_sb[:, :], d_bias[:, :])
            ones8 = res.tile([128, 256], mybir.dt.float8e4)
            nc.vector.memset(ones8[:, :], 1.0)

            for rep in range(reps):
                x8 = x8p.tile([128, NKP, 2, MQ], mybir.dt.float8e4, name="x8")
                s32p = accp.tile([128, MQ], mybir.dt.float32, name="s32p")
                sdve = accp.tile([128, MQ], mybir.dt.float32, name="sdve")
                rs = accp.tile([128, MQ], mybir.dt.float32, name="rs")

                psums = {}

                def emit_x_pipeline(kp, x8=x8, s32p=s32p, sdve=sdve):
                    st = xsp.tile([128, 2, MQ], mybir.dt.float16, name="xstage")
                    nc.sync.dma_start(
                        st[:, :, :], d_xt[2 * kp * 128 : (2 * kp + 2) * 128, :]
                    )
                    # fp16 -> fp8 convert (whole pair in one op)
                    if kp % DVE_X_PERIOD == DVE_X_PERIOD - 1:
                        nc.vector.tensor_scalar(
                            x8[:, kp, :, :], st[:, :, :], 0.0, None,
                            op0=mybir.AluOpType.add,
                        )
                    else:
                        nc.scalar.activation(
                            x8[:, kp, :, :], st[:, :, :],
                            mybir.ActivationFunctionType.Copy,
                            bias=0.0, scale=1.0,
                        )
                    # pair-reduce on DVE (fp16) -> frees staging quickly
                    tp = tpp.tile([128, MQ], mybir.dt.float16, name="tpair")
                    nc.vector.tensor_tensor(
                        tp[:, :], st[:, 0, :], st[:, 1, :], op=mybir.AluOpType.add
                    )
                    # S16 chains: Pool for early kps, DVE for the rest
                    if kp < POOL_S_PAIRS:
                        if kp == 0:
                            nc.gpsimd.tensor_copy(s32p[:, :], tp[:, :])
                        else:
                            nc.gpsimd.tensor_tensor(
                                s32p[:, :], s32p[:, :], tp[:, :],
                                op=mybir.AluOpType.add,
                            )
                    else:
                        if kp == POOL_S_PAIRS:
                            nc.vector.tensor_scalar(
                                sdve[:, :], tp[:, :], 0.0, None,
                                op0=mybir.AluOpType.add,
                            )
                        else:
                            nc.vector.tensor_tensor(
                                sdve[:, :], sdve[:, :], tp[:, :],
                                op=mybir.AluOpType.add,
                            )

                def emit_decode(wave_i, mt, s):
                    c0, c1 = WAVE_COLS[wave_i]
                    cols = c1 - c0
                    t = twp.tile([128, 2, cols], mybir.dt.int16, name="twt")
                    if s == 0:
                        nc.vector.tensor_scalar(
                            t[:, :, :], ht_sb[:, mt, :, c0:c1], 0x0303, None,
                            op0=mybir.AluOpType.bitwise_and,
                        )
                    else:
                        nc.vector.tensor_scalar(
                            t[:, :, :], ht_sb[:, mt, :, c0:c1], 2 * s, 0x0303,
                            op0=mybir.AluOpType.logical_shift_right,
                            op1=mybir.AluOpType.bitwise_and,
                        )
                    nc.vector.tensor_scalar(
                        t[:, :, :], t[:, :, :], 0x5050, None,
                        op0=mybir.AluOpType.add,
                    )
                    return t.bitcast(mybir.dt.float8e4)  # [128, 2, 2*cols]

                for wave_i, wave in enumerate(WAVES):
                    c0, _ = WAVE_COLS[wave_i]
                    for u in wave:
                        psums[u] = psp.tile(
                            [128, MQ], mybir.dt.float32, name="psum", tag="psum"
                        )
                    t8 = None
                    for kp in range(NKP):
                        mt, s, h = kp // 8, (kp % 8) // 2, kp % 2
                        if wave_i == 0:
                            emit_x_pipeline(kp)
                        if h == 0:
                            t8 = emit_decode(wave_i, mt, s)
                        for u in wave:
                            if u == "S8":
                                stat = ones8[:, :]
                            else:
                                rel0 = u * 128 - c0
                                stat = t8[:, h, 2 * rel0 : 2 * rel0 + 256]
                            nc.tensor.matmul(
                                psums[u][:, :],
                                stat,
                                x8[:, kp, :, :],
                                start=(kp == 0),
                                stop=(kp == NKP - 1),
                                perf_mode=mybir.MatmulPerfMode.DoubleRowSwInterleave,
                            )
                    if wave_i == 0:
                        # rs = (4/3)*allreduce(s32p [+ sdve]) - (28/3)*S8
                        # (TTs on Pool, psum-reads/scales on ACT: keep DVE free)
                        if POOL_S_PAIRS < NKP:
                            nc.gpsimd.tensor_tensor(
                                sdve[:, :], sdve[:, :], s32p[:, :],
                                op=mybir.AluOpType.add,
                            )
                        else:
                            nc.gpsimd.tensor_copy(sdve[:, :], s32p[:, :])
                        nc.gpsimd.partition_all_reduce(
                            s32p[:, :], sdve[:, :], channels=128,
                            reduce_op=bass_isa.ReduceOp.add,
                        )
                        t1 = pap.tile([128, MQ], mybir.dt.float32, name="t1")
                        nc.scalar.activation(
                            t1[:, :], psums["S8"][:, :],
                            mybir.ActivationFunctionType.Copy,
                            bias=0.0, scale=C283,
                        )
                        nc.scalar.activation(
                            rs[:, :], s32p[:, :],
                            mybir.ActivationFunctionType.Copy,
                            bias=0.0, scale=C43,
                        )
                        nc.gpsimd.tensor_tensor(
                            rs[:, :], rs[:, :], t1[:, :],
                            op=mybir.AluOpType.subtract,
                        )
                    # evac: part = psum + bias (ACT, frees bank fast), then
                    # final y = fp16(part + rs) on Pool, out DMA from Pool DGE
                    for u in wave:
                        if u == "S8":
                            continue
                        part = pap.tile([128, MQ], mybir.dt.float32, name="partial")
                        nc.scalar.activation(
                            part[:, :], psums[u][:, :],
                            mybir.ActivationFunctionType.Identity,
                            bias=bias_sb[:, u : u + 1], scale=1.0,
                        )
                        y = otp.tile([128, MQ], mybir.dt.float16, name="yout")
                        nc.gpsimd.tensor_tensor(
                            y[:, :], part[:, :], rs[:, :], op=mybir.AluOpType.add
                        )
                        nc.gpsimd.dma_start(d_out[u * 128 : (u + 1) * 128, :], y[:, :])
    nc.finalize()
    return nc


def get_program(reps=1):
    key = reps
    if key not in _PROGRAM_CACHE:
        _PROGRAM_CACHE[key] = _build_program(reps)
    return _PROGRAM_CACHE[key]


def prep_inputs(x, packed_w, bias, ncores=NCORES):
    """Pure-layout host prep (transpose / plane reorder / int16 byte view)."""
    x = np.asarray(x, dtype=np.float16)
    packed_w = np.asarray(packed_w, dtype=np.int32)
    bias32 = np.asarray(bias, dtype=np.float32)

    xps = []
    for mh in range(MSPLIT):
        xt = np.ascontiguousarray(x[mh * MQ : (mh + 1) * MQ].T)  # (KQ, MQ)
        x4 = xt.reshape(NMT, 128, 16, MQ).transpose(0, 2, 1, 3)  # [mt, f, 128, MQ]
        xp = x4[:, _FIELD_ORDER].reshape(KQ, MQ)
        # staging DMA maps DRAM row 2p+i of each 256-row block to
        # st[p, i, :]; interleave each plane pair row-by-row to match
        xp = np.ascontiguousarray(
            xp.reshape(NKP, 2, 128, MQ).transpose(0, 2, 1, 3).reshape(KQ, MQ)
        )
        xps.append(xp)

    hts, bls = [], []
    for nq in range(NSPLIT):
        pwc = packed_w[nq * NLOC : (nq + 1) * NLOC]  # (NLOC, NW)
        # reverse rows within each 128-row block (SwInterleave col order)
        pwcr = pwc.reshape(NJT, 128, NW)[:, ::-1].reshape(NLOC, NW)
        h0 = np.ascontiguousarray(pwcr.T).view(np.int16)  # (NW, 2*NLOC), n-major pairs
        ht = np.ascontiguousarray(
            h0.reshape(NW, NLOC, 2).transpose(0, 2, 1).reshape(NW, 2 * NLOC)
        )
        hts.append(ht)
        bls.append(
            np.ascontiguousarray(
                bias32[nq * NLOC : (nq + 1) * NLOC].reshape(NJT, 128).T
            )
        )

    in_maps = []
    for c in range(ncores):
        mh, nq = c // NSPLIT, c % NSPLIT
        in_maps.append({"xt": xps[mh], "ht": hts[nq], "bias": bls[nq]})
    return in_maps


def assemble_output(outs):
    y = np.empty((M_FULL, N_FULL), dtype=np.float16)
    for c, o in enumerate(outs):
        mh, nq = c // NSPLIT, c % NSPLIT
        y[mh * MQ : (mh + 1) * MQ, nq * NLOC : (nq + 1) * NLOC] = o.T
    return y


def kernel(x, packed_w, bias):
    from concourse.bass_utils import run_bass_kernel_spmd

    nc = get_program()
    in_maps = prep_inputs(np.asarray(x), np.asarray(packed_w), np.asarray(bias))
    res = run_bass_kernel_spmd(nc, in_maps, core_ids=list(range(NCORES)))
    return assemble_output([r["out"] for r in res.results])
